# revision 1
# baseline (speedup 1.0000x reference)
"""Trainium2 Bass kernel for windowed local attention (8x8 windows).

Full computation (reference):
  x [B=8, C=192, H=256, W=256] -> window partition (8x8) -> per-window:
  qkv = w_qkv @ win + b_qkv ; attn = softmax(q^T k / sqrt(C)) ;
  out = v @ attn^T ; y = w_proj @ out + b_proj -> window reverse.

Sharding: data-parallel over batch. Core b handles image b (32 window-rows
("bands") of 32 windows each). Weights replicated. All matmul datapaths
bf16 (x cast on host), PSUM/softmax fp32.

Key algebraic restructure (vs the straightforward pipeline):
  y_win = w_proj @ (v @ P^T) + b_proj  with v = wv@x + bv collapses to
  y_win = (wu @ x_win) @ P^T + bpp  where wu = w_proj @ wv (host-folded)
  and bpp = b_proj + w_proj@bv (softmax rows sum to 1). So v, the
  attention output, and the separate projection matmul are never
  materialized: per window pair we compute wT = (wu@x_pair)^T directly
  in token-major form (x as the stationary operand, wuT moving), then
  y = wT^T @ P^T per pair. This kills three PSUM->SBUF copy streams
  (v, v^T, attn-out) and the separate proj matmuls.

Per-band pipeline (band = [C, 8, W] slab, 4 groups of 4 window-pairs):
  A: q,k band matmuls (C=192 contraction as 128+64), PSUM->SBUF copies
     do the window-major token reorder; q bias on ACT, k bias on DVE
     (tensor_scalar_add) to balance engines.
  W: wT = (wu@x_pair)^T per pair, 2 pairs per PSUM bank, DVE copy.
  C: scores for window PAIRS (2x64 tokens = 128 partitions; off-diagonal
     cross-window blocks are garbage, zeroed after exp).
  softmax: exp on ACT (scores ~ N(0,1), exp safe in fp32); garbage
     blocks zeroed by GPSIMD memsets; row-sum + reciprocal + scale on DVE.
  D: attn^T via identity matmul on the PE.
  Y: y = wT^T @ attn^T (pair-blocked; zeros kill cross terms), then
     final copy with bpp bias scatters into the output band (ACT/DVE
     split per group).

The qk scale is folded into Wq, bq on the host.
"""

import os
import sys

import numpy as np

if "/opt/trn_rl_repo" not in sys.path:
    sys.path.insert(0, "/opt/trn_rl_repo")

C = 192
WS = 8
S = WS * WS  # 64 tokens per window


def build_program(n_bands=32, width=256):
    import concourse.bass as bass  # noqa: F401
    import concourse.tile as tile
    from concourse import bacc, mybir

    f32 = mybir.dt.float32
    bf16 = mybir.dt.bfloat16
    GPB = width // 64  # groups per band (4 pairs = 8 windows each)
    NW = width // WS  # windows per band

    nc = bacc.Bacc("TRN2", target_bir_lowering=False, debug=False)

    Hn = n_bands * WS
    x = nc.dram_tensor("x", [C, Hn, width], bf16, kind="ExternalInput").ap()
    y = nc.dram_tensor("y", [C, Hn, width], f32, kind="ExternalOutput").ap()
    wqT = nc.dram_tensor("wqT", [C, C], bf16, kind="ExternalInput").ap()
    wkT = nc.dram_tensor("wkT", [C, C], bf16, kind="ExternalInput").ap()
    wuT = nc.dram_tensor("wuT", [C, C], bf16, kind="ExternalInput").ap()
    bq = nc.dram_tensor("bq", [C, 1], f32, kind="ExternalInput").ap()
    bk = nc.dram_tensor("bk", [C, 1], f32, kind="ExternalInput").ap()
    bpp = nc.dram_tensor("bpp", [C, 1], f32, kind="ExternalInput").ap()
    eye = nc.dram_tensor("eye", [128, 128], bf16, kind="ExternalInput").ap()

    Ident = mybir.ActivationFunctionType.Identity
    Exp = mybir.ActivationFunctionType.Exp
    AX = mybir.AxisListType.X

    def blk(t2d, p):
        # [P, 512] -> [P, 128] block p
        return t2d.rearrange("p (pr n) -> p pr n", pr=4)[:, p]

    from contextlib import ExitStack

    with tile.TileContext(nc) as tc, ExitStack() as ctx:
        cp = ctx.enter_context(tc.tile_pool(name="consts", bufs=1))
        xp = ctx.enter_context(tc.tile_pool(name="xbands", bufs=2))
        qkp = ctx.enter_context(tc.tile_pool(name="qk", bufs=2))
        wtsp = ctx.enter_context(tc.tile_pool(name="wts", bufs=6))
        ep = ctx.enter_context(tc.tile_pool(name="e", bufs=3))
        atsp = ctx.enter_context(tc.tile_pool(name="ats", bufs=3))
        rp = ctx.enter_context(tc.tile_pool(name="r", bufs=3))
        dgp = ctx.enter_context(tc.tile_pool(name="dg", bufs=3))
        fbp = ctx.enter_context(tc.tile_pool(name="fb", bufs=2))
        pp_main = ctx.enter_context(tc.tile_pool(name="pp_main", bufs=3, space="PSUM"))
        pp_sd = ctx.enter_context(tc.tile_pool(name="pp_sd", bufs=3, space="PSUM"))
        pp_wt = ctx.enter_context(tc.tile_pool(name="pp_wt", bufs=2, space="PSUM"))

        # ---- constants ----
        def const_2d(name, src, p0, p1, cols, dt=bf16):
            t = cp.tile([p1 - p0, cols], dt, tag=name)
            nc.sync.dma_start(out=t[:], in_=src[p0:p1, 0:cols])
            return t

        wq1 = const_2d("wq1", wqT, 0, 128, C)
        wq2 = const_2d("wq2", wqT, 128, 192, C)
        wk1 = const_2d("wk1", wkT, 0, 128, C)
        wk2 = const_2d("wk2", wkT, 128, 192, C)
        wu1 = const_2d("wu1", wuT, 0, 128, C)
        wu2 = const_2d("wu2", wuT, 128, 192, C)
        bq1 = const_2d("bq1", bq, 0, 128, 1, f32)
        bq2 = const_2d("bq2", bq, 128, 192, 1, f32)
        bk1 = const_2d("bk1", bk, 0, 128, 1, f32)
        bk2 = const_2d("bk2", bk, 128, 192, 1, f32)
        bp1 = const_2d("bp1", bpp, 0, 128, 1, f32)
        bp2 = const_2d("bp2", bpp, 128, 192, 1, f32)
        ident = const_2d("ident", eye, 0, 128, 128)

        ri2 = 512 // width  # band rows per chunk

        def alloc_band(hw):
            st = {}
            st["hw"] = hw
            xb1 = xp.tile([128, 8, width], bf16, tag="xb1")
            nc.sync.dma_start(out=xb1[:], in_=x[0:128, hw * 8:(hw + 1) * 8, :])
            xb2 = xp.tile([64, 8, width], bf16, tag="xb2")
            nc.sync.dma_start(out=xb2[:], in_=x[128:192, hw * 8:(hw + 1) * 8, :])
            st["xf"] = (xb1[:].rearrange("p i w -> p (i w)"),
                        xb2[:].rearrange("p i w -> p (i w)"))
            # window-major copy of x (on idle GPSIMD): gives the W-step a
            # contiguous stationary slice (LDWEIGHTS APs allow only one
            # free dim)
            xw1 = xp.tile([128, NW * 64], bf16, tag="xw1", name="xw1")
            xw2 = xp.tile([64, NW * 64], bf16, tag="xw2", name="xw2")
            for src, dst in ((xb1, xw1), (xb2, xw2)):
                nc.vector.tensor_copy(
                    dst[:].rearrange("p (ww i j) -> p ww i j", ww=NW, i=8, j=8),
                    src[:].rearrange("p i (ww j) -> p ww i j", ww=NW, j=WS))
            st["xw"] = (xw1, xw2)
            st["q1"] = qkp.tile([128, NW * 64], bf16, tag="q1", name="q1")
            st["q2"] = qkp.tile([64, NW * 64], bf16, tag="q2", name="q2")
            st["k1"] = qkp.tile([128, NW * 64], bf16, tag="k1", name="k1")
            st["k2"] = qkp.tile([64, NW * 64], bf16, tag="k2", name="k2")
            # window-major views (the A copies scatter into token order)
            st["wm"] = {
                n: st[n][:].rearrange("p (ww i j) -> p i ww j", ww=NW, i=8, j=8)
                for n in ("q1", "q2", "k1", "k2")
            }
            fb1 = fbp.tile([128, 8, width], f32, tag="fb1")
            fb2 = fbp.tile([64, 8, width], f32, tag="fb2")
            st["fb"] = (fb1, fb2)
            st["fr"] = (fb1[:].rearrange("p i (gg w8 j) -> p gg w8 i j",
                                         gg=GPB, w8=8, j=8),
                        fb2[:].rearrange("p i (gg w8 j) -> p gg w8 i j",
                                         gg=GPB, w8=8, j=8))
            return st

        def a_substeps(st):
            """Yield after each (chunk, tensor) substep: 4 matmuls + 2
            copies. q copies ride ACT (with bias), k copies ride DVE, and
            chunks alternate q/k so both engines get fed every iteration."""
            xf1, xf2 = st["xf"]
            for ncnk in range(width * 8 // 512):
                rhs1 = xf1[:, ncnk * 512:(ncnk + 1) * 512]
                rhs2 = xf2[:, ncnk * 512:(ncnk + 1) * 512]
                for w1, w2, n1, n2, b1, b2, on_act in (
                        (wq1, wq2, "q1", "q2", bq1, bq2, True),
                        (wk1, wk2, "k1", "k2", bk1, bk2, False)):
                    p1 = pp_main.tile([128, 512], f32, tag="main")
                    p2 = pp_main.tile([64, 512], f32, tag="main")
                    for mlo, mhi, op in ((0, 128, p1), (128, 192, p2)):
                        nc.tensor.matmul(op[:], w1[:, mlo:mhi], rhs1,
                                         start=True, stop=False)
                        nc.tensor.matmul(op[:], w2[:, mlo:mhi], rhs2,
                                         start=False, stop=True)
                    for ps, n, bias in ((p1, n1, b1), (p2, n2, b2)):
                        outv = st["wm"][n][:, ncnk * ri2:(ncnk + 1) * ri2]
                        if on_act:
                            nc.scalar.activation(outv, ps[:], Ident,
                                                 bias=bias[:, 0:1])
                        else:
                            nc.vector.tensor_scalar_add(outv, ps[:],
                                                        bias[:, 0:1])
                    yield

        def stage_W(st, g):
            # wT = (wu @ x_pair)^T directly (x stationary)
            xw1, xw2 = st["xw"]
            halves = []
            for half in range(2):
                wtp = pp_wt.tile([128, 2, C], f32, tag="wt")
                for pi in range(2):
                    off = (g * 8 + 2 * (2 * half + pi)) * 64
                    nc.tensor.matmul(wtp[:, pi], xw1[:, off:off + 128],
                                     wu1[:], start=True, stop=False)
                    nc.tensor.matmul(wtp[:, pi], xw2[:, off:off + 128],
                                     wu2[:], start=False, stop=True)
                wts = wtsp.tile([128, 2, C], bf16, tag="wts")
                nc.scalar.activation(wts[:], wtp[:], Ident)
                halves.append(wts)
            return halves

        def stage_C(st, g):
            # pair-blocked scores; off-diagonal 64-blocks are garbage
            # (never read: exp only touches the diagonal blocks)
            q1, q2, k1, k2 = st["q1"], st["q2"], st["k1"], st["k2"]
            scp = pp_sd.tile([128, 512], f32, tag="sd")
            for p in range(4):
                off = (g * 8 + 2 * p) * 64
                nc.tensor.matmul(blk(scp[:], p), q1[:, off:off + 128],
                                 k1[:, off:off + 128],
                                 start=True, stop=False)
                nc.tensor.matmul(blk(scp[:], p), q2[:, off:off + 128],
                                 k2[:, off:off + 128],
                                 start=False, stop=True)
            return scp

        # ---- flat cross-band software pipeline ----
        # iteration (hw, g) issues: feeder substeps of band hw+1's A phase,
        # C(g+1), softmax(g), D(g), W(g+1), Y(g), final copies (g).
        st = alloc_band(0)
        for _ in a_substeps(st):
            pass
        wts_half = stage_W(st, 0)
        scp = stage_C(st, 0)
        st_next = None
        feeder = None
        for hw in range(n_bands):
            for g in range(GPB):
                if g == 0 and hw + 1 < n_bands:
                    st_next = alloc_band(hw + 1)
                    feeder = a_substeps(st_next)
                last_g = g == GPB - 1
                if feeder is not None:
                    next(feeder, None)
                    if last_g:
                        next(feeder, None)  # finish A(hw+1) before C(next,0)
                if not last_g:
                    scp_next = stage_C(st, g + 1)
                elif st_next is not None:
                    scp_next = stage_C(st_next, 0)
                else:
                    scp_next = None

                # ---- softmax(g): exp of the valid diagonal blocks only.
                # The garbage regions of the pool-recycled e buffers are
                # zeroed once (first band) and never written again, so
                # row-sums over the full width and the transposed attn
                # stay correct (no max subtraction: scores ~ N(0,1)). ----
                e = ep.tile([128, 4, 128], bf16, tag="e")
                # zero the cross-window garbage blocks (disjoint from the
                # exp writes below, so these run in parallel on GPSIMD)
                nc.gpsimd.memset(e[0:64, :, 64:128], 0.0)
                nc.gpsimd.memset(e[64:128, :, 0:64], 0.0)
                scv = scp[:].rearrange("p (pr n) -> p pr n", pr=4)
                nc.scalar.activation(e[0:64, :, 0:64], scv[0:64, :, 0:64],
                                     Exp)
                nc.scalar.activation(e[64:128, :, 64:128],
                                     scv[64:128, :, 64:128], Exp)
                rs = rp.tile([128, 4], f32, tag="rs")
                nc.vector.reduce_sum(rs[0:64, :], e[0:64, :, 0:64], axis=AX)
                nc.vector.reduce_sum(rs[64:128, :], e[64:128, :, 64:128],
                                     axis=AX)
                ri = rp.tile([128, 4], f32, tag="ri")
                nc.vector.reciprocal(ri[:], rs[:])
                # per-pair r-scaled identity on GPSIMD: D[s, :] = r[s]*e_s
                dg = dgp.tile([128, 4, 128], bf16, tag="dg")
                for p in range(4):
                    nc.vector.tensor_scalar_mul(dg[:, p], ident[:],
                                                ri[:, p:p + 1])

                # ---- D(g): normalized attn^T in one PE op per pair:
                # P^T = e^T @ diag(r)  (transpose + row-normalize fused) ----
                atp = pp_sd.tile([128, 4, 128], f32, tag="sd")
                for p in range(4):
                    nc.tensor.matmul(atp[:, p], e[:, p, :], dg[:, p])
                ats = atsp.tile([128, 4, 128], bf16, tag="ats")
                nc.vector.tensor_copy(ats[:], atp[:])

                if feeder is not None and not last_g:
                    next(feeder, None)
                if not last_g:
                    wts_next = stage_W(st, g + 1)
                elif st_next is not None:
                    wts_next = stage_W(st_next, 0)
                else:
                    wts_next = None

                # ---- Y(g): y = wT^T @ attn^T (pair-blocked; zeros in the
                # garbage blocks of ats kill the cross-window terms) ----
                yp1 = pp_main.tile([128, 512], f32, tag="main")
                yp2 = pp_main.tile([64, 512], f32, tag="main")
                for p in range(4):
                    wts = wts_half[p // 2]
                    pi = p % 2
                    nc.tensor.matmul(blk(yp1[:], p), wts[:, pi, 0:128],
                                     ats[:, p, :])
                    nc.tensor.matmul(blk(yp2[:], p), wts[:, pi, 128:192],
                                     ats[:, p, :])

                # ---- final copy with bias, scattered into band buffer ----
                fr1, fr2 = st["fr"]
                nc.scalar.activation(fr1[:, g], yp1[:], Ident, bias=bp1[:, 0:1])
                nc.vector.tensor_scalar_add(fr2[:, g], yp2[:], bp2[:, 0:1])

                wts_half = wts_next
                scp = scp_next

            fb1, fb2 = st["fb"]
            nc.sync.dma_start(out=y[0:128, hw * 8:(hw + 1) * 8, :], in_=fb1[:])
            nc.sync.dma_start(out=y[128:192, hw * 8:(hw + 1) * 8, :], in_=fb2[:])
            st = st_next
            st_next = None
            feeder = None

    nc.compile()
    return nc


def prep_weights(w_qkv, b_qkv, w_proj, b_proj):
    import ml_dtypes

    bf16 = ml_dtypes.bfloat16
    scale = np.float32(C ** -0.5)
    w_qkv = np.asarray(w_qkv, dtype=np.float32)
    b_qkv = np.asarray(b_qkv, dtype=np.float32)
    w_proj = np.asarray(w_proj, dtype=np.float32)
    b_proj = np.asarray(b_proj, dtype=np.float32)
    wq, wk, wv = w_qkv[0:C], w_qkv[C:2 * C], w_qkv[2 * C:3 * C]
    wu = w_proj @ wv
    return {
        "wqT": np.ascontiguousarray((wq * scale).T).astype(bf16),
        "wkT": np.ascontiguousarray(wk.T).astype(bf16),
        "wuT": np.ascontiguousarray(wu.T).astype(bf16),
        "bq": np.ascontiguousarray((b_qkv[0:C] * scale).reshape(C, 1)),
        "bk": np.ascontiguousarray(b_qkv[C:2 * C].reshape(C, 1)),
        "bpp": np.ascontiguousarray(
            (b_proj + w_proj @ b_qkv[2 * C:3 * C]).reshape(C, 1)),
        "eye": np.eye(128, dtype=np.float32).astype(bf16),
    }


_PROGRAM_CACHE = {}


def get_program(n_bands, width=256):
    key = (n_bands, width)
    if key not in _PROGRAM_CACHE:
        _PROGRAM_CACHE[key] = build_program(n_bands, width)
    return _PROGRAM_CACHE[key]


def make_in_maps(x, w_qkv, b_qkv, w_proj, b_proj):
    import ml_dtypes

    x = np.asarray(x, dtype=np.float32).astype(ml_dtypes.bfloat16)
    wts = prep_weights(w_qkv, b_qkv, w_proj, b_proj)
    return [{"x": np.ascontiguousarray(x[b]), **wts} for b in range(x.shape[0])]


def assemble_output(results):
    out = np.stack([results[b]["y"] for b in range(len(results))], axis=0)
    return out.astype(np.float32)


def kernel(x, w_qkv, b_qkv, w_proj, b_proj):
    from concourse.bass_utils import run_bass_kernel_spmd

    x = np.asarray(x, dtype=np.float32)
    B, c, H, W = x.shape
    assert c == C
    nc = get_program(H // WS, W)
    in_maps = make_in_maps(x, w_qkv, b_qkv, w_proj, b_proj)
    res = run_bass_kernel_spmd(nc, in_maps, core_ids=list(range(B)))
    return assemble_output(res.results)



# revision 2
# speedup vs baseline: 68.8199x; 68.8199x over previous
"""Trainium2 Bass kernel for windowed local attention (8x8 windows), v6.

Full computation (reference):
  x [B=8, C=192, H=256, W=256] -> window partition (8x8) -> per-window:
  qkv = w_qkv @ win + b_qkv ; attn = softmax(q^T k / sqrt(C)) ;
  out = v @ attn^T ; y = w_proj @ out + b_proj -> window reverse.

Sharding: data-parallel over batch; core b handles image b (32 bands of
8 rows x 256 cols = 32 windows each).  Weights replicated.  The host
ships x already in window-major token order [C, band, window, i, j], so
one SBUF tile per band serves as the G-phase moving operand, the
W-stage stationary, and the C-stage moving operand with no on-chip
reorder, and every PSUM evacuation is a contiguous copy.

Algebraic restructure:
  - Scores: S = q^T k = x^T M x with M = scale * Wq^T Wk folded on the
    host; q and k are never computed.  The score bias terms reduce to
    a_t = (scale * Wk^T bq)^T x_t (s-indexed terms are constant along
    the softmax axis and drop out exactly); a_t rides as output row 192
    of the G phase (g = [M; alpha^T] x) and enters the scores through a
    persistent ones-row in the C-stage operands.
  - TRANSPOSED scores S^T[t,s] = g^T x, so no attention-transpose
    matmul is needed and the softmax normalizer is columnwise:
    per-pair ones-vector matmuls give the colsum TRANSPOSED [s, 4],
    which makes the reciprocal a cheap 4-free-element DVE op; the
    result is transposed back and broadcast by K=1 matmuls on the PE
    (no transcendental-table switches, no iterative 512-wide recip).
  - y = Wu (x P^T) + bpp with Wu = Wproj Wv, bpp = bproj + Wproj bv:
    wT = (Wu x_pair)^T per pair (x stationary), with bpp folded into wT
    through the same ones-row (softmax columns of P^T sum to 1); the
    final PSUM evacuations are pure copies.  Cross-window terms in Y's
    K=128 two-window packing are killed by zeros in P^T's off-diagonal
    blocks (eT garbage is re-zeroed by GPSIMD memsets each group).

PSUM plan (8 banks): G 2-deep; {scores, colsum, transpose, broadcast}
share one phase-stable 2-deep rotation; W 2-deep; Y [128,2,512] 1-deep.
Shallow PSUM rotations stall the PE on write-after-read and keep the
HAM clock gate at 1.2 GHz -- depth here is what buys the 2.4 GHz clock.
"""

import os
import sys

import numpy as np

if "/opt/trn_rl_repo" not in sys.path:
    sys.path.insert(0, "/opt/trn_rl_repo")

C = 192
WS = 8
S = WS * WS  # 64 tokens per window


def build_program(n_bands=32, width=256):
    import concourse.bass as bass  # noqa: F401
    import concourse.tile as tile
    from concourse import bacc, mybir

    f32 = mybir.dt.float32
    bf16 = mybir.dt.bfloat16
    GPB = width // 64  # groups per band (4 pairs = 8 windows each)
    NW = width // WS  # windows per band
    NT = NW * 64  # tokens per band

    nc = bacc.Bacc("TRN2", target_bir_lowering=False, debug=False)

    # x is window-major: [C, band, window, i, j] flattened to [C, nb, NT]
    x = nc.dram_tensor("x", [C, n_bands, NT], bf16, kind="ExternalInput").ap()
    y = nc.dram_tensor("y", [C, n_bands * 8, width], f32,
                       kind="ExternalOutput").ap()
    wgk1 = nc.dram_tensor("wgk1", [128, 193], bf16, kind="ExternalInput").ap()
    wgk2 = nc.dram_tensor("wgk2", [64, 193], bf16, kind="ExternalInput").ap()
    wut1 = nc.dram_tensor("wut1", [128, C], bf16, kind="ExternalInput").ap()
    wut2 = nc.dram_tensor("wut2", [65, C], bf16, kind="ExternalInput").ap()
    eye = nc.dram_tensor("eye", [128, 128], bf16, kind="ExternalInput").ap()

    Ident = mybir.ActivationFunctionType.Identity
    Exp = mybir.ActivationFunctionType.Exp
    Mult = mybir.AluOpType.mult

    def blk(t2d, p):
        # [P, 512] -> [P, 128] block p
        return t2d.rearrange("p (pr n) -> p pr n", pr=4)[:, p]

    from contextlib import ExitStack

    with tile.TileContext(nc) as tc, ExitStack() as ctx:
        cp = ctx.enter_context(tc.tile_pool(name="consts", bufs=1))
        xp = ctx.enter_context(tc.tile_pool(name="xbands", bufs=2))
        gsb = ctx.enter_context(tc.tile_pool(name="gsb", bufs=2))
        ep = ctx.enter_context(tc.tile_pool(name="e", bufs=2))
        rp = ctx.enter_context(tc.tile_pool(name="r", bufs=2))
        ptp = ctx.enter_context(tc.tile_pool(name="ptp", bufs=2))
        wtsp = ctx.enter_context(tc.tile_pool(name="wts", bufs=6))
        fbp = ctx.enter_context(tc.tile_pool(name="fb", bufs=2))
        pp_g = ctx.enter_context(tc.tile_pool(name="pp_g", bufs=2, space="PSUM"))
        pp_s = ctx.enter_context(tc.tile_pool(name="pp_s", bufs=1, space="PSUM"))
        pp_n = ctx.enter_context(tc.tile_pool(name="pp_n", bufs=1, space="PSUM"))
        pp_w = ctx.enter_context(tc.tile_pool(name="pp_w", bufs=2, space="PSUM"))
        pp_y = ctx.enter_context(tc.tile_pool(name="pp_y", bufs=1, space="PSUM"))

        # ---- constants ----
        def const_2d(name, src, p0, p1, cols, dt=bf16):
            t = cp.tile([p1 - p0, cols], dt, tag=name, name=name)
            nc.sync.dma_start(out=t[:], in_=src[p0:p1, 0:cols])
            return t

        wg1 = const_2d("wg1", wgk1, 0, 128, 193)
        wg2 = const_2d("wg2", wgk2, 0, 64, 193)
        wu1 = const_2d("wu1", wut1, 0, 128, C)
        wu2 = const_2d("wu2", wut2, 0, 65, C)
        ident = const_2d("ident", eye, 0, 128, 128)
        onescol = cp.tile([128, 1], bf16, tag="onescol", name="onescol")
        nc.gpsimd.memset(onescol[:], 1.0)
        onesrow = cp.tile([1, 128], bf16, tag="onesrow", name="onesrow")
        nc.gpsimd.memset(onesrow[:], 1.0)

        # xw2o carries a persistent ones row at partition 64 (a-term and
        # bpp folding); manual rotation because pool generations would
        # re-tensor the slot and trip the subtile dependency checker.
        xw2o_tiles = []
        for i in range(2):
            xw2o = cp.tile([65, NT], bf16, tag=f"xw2o{i}", name=f"xw2o{i}")
            nc.gpsimd.memset(xw2o[64:65, :], 1.0)
            xw2o_tiles.append(xw2o)

        def alloc_band(hw):
            st = {}
            st["hw"] = hw
            xw1 = xp.tile([128, NT], bf16, tag="xw1", name="xw1")
            nc.sync.dma_start(out=xw1[:], in_=x[0:128, hw])
            xw2o = xw2o_tiles[hw % 2]
            nc.sync.dma_start(out=xw2o[0:64, :], in_=x[128:192, hw])
            st["xw1"], st["xw2o"] = xw1, xw2o
            fb1 = fbp.tile([128, 8, width], f32, tag="fb1", name="fb1")
            fb2 = fbp.tile([64, 8, width], f32, tag="fb2", name="fb2")
            st["fb"] = (fb1, fb2)
            st["fr"] = (fb1[:].rearrange("p i (gg w8 j) -> p gg w8 i j",
                                         gg=GPB, w8=8, j=8),
                        fb2[:].rearrange("p i (gg w8 j) -> p gg w8 i j",
                                         gg=GPB, w8=8, j=8))
            g1 = gsb.tile([128, NT], bf16, tag="g1", name="g1")
            g2a = gsb.tile([65, NT], bf16, tag="g2a", name="g2a")
            st["g1"], st["g2a"] = g1, g2a
            return st

        def g_substeps(st):
            """4 substeps: (block, chunk-pair).  x is window-major, so a
            512-token chunk c is exactly group c and the evacuations are
            contiguous copies into the g tiles."""
            xw1, xw2o = st["xw1"], st["xw2o"]
            for b in (0, 1):
                c0, c1, P = (0, 128, 128) if b == 0 else (128, 193, 65)
                gt = st["g1"] if b == 0 else st["g2a"]
                for j2 in range(2):
                    ce, co = 2 * j2, 2 * j2 + 1
                    pe = pp_g.tile([P, 512], f32, tag="gp", name="pe")
                    po = pp_g.tile([P, 512], f32, tag="gp", name="po")
                    nc.tensor.matmul(pe[:], wg1[:, c0:c1],
                                     xw1[:, ce * 512:(ce + 1) * 512],
                                     start=True, stop=False)
                    nc.tensor.matmul(pe[:], wg2[:, c0:c1],
                                     xw2o[0:64, ce * 512:(ce + 1) * 512],
                                     start=False, stop=True)
                    nc.tensor.matmul(po[:], wg1[:, c0:c1],
                                     xw1[:, co * 512:(co + 1) * 512],
                                     start=True, stop=False)
                    nc.tensor.matmul(po[:], wg2[:, c0:c1],
                                     xw2o[0:64, co * 512:(co + 1) * 512],
                                     start=False, stop=True)
                    nc.scalar.activation(gt[0:P, ce * 512:(ce + 1) * 512],
                                         pe[:], Ident)
                    nc.vector.tensor_copy(gt[0:P, co * 512:(co + 1) * 512],
                                          po[:])
                    yield

        def stage_WC_half(st, g, half, scp):
            """Fused W (wT = (Wu x_pair)^T + bpp via ones-row) and
            C (S^T = g^T x + a (x) 1 via the same ones-row), 2 pairs."""
            xw1, xw2o = st["xw1"], st["xw2o"]
            g1, g2a = st["g1"], st["g2a"]
            wtp = pp_w.tile([128, 2, C], f32, tag="wt", name="wtp")
            for pi in (0, 1):
                p = 2 * half + pi
                off = (g * 8 + 2 * p) * 64
                nc.tensor.matmul(wtp[:, pi], xw1[:, off:off + 128],
                                 wu1[:], start=True, stop=False)
                nc.tensor.matmul(wtp[:, pi], xw2o[0:65, off:off + 128],
                                 wu2[:], start=False, stop=True)
                nc.tensor.matmul(blk(scp[:], p), g1[:, off:off + 128],
                                 xw1[:, off:off + 128],
                                 start=True, stop=False)
                nc.tensor.matmul(blk(scp[:], p), g2a[0:65, off:off + 128],
                                 xw2o[0:65, off:off + 128],
                                 start=False, stop=True)
            wts = wtsp.tile([128, 2, C], bf16, tag="wts", name="wts")
            if half == 0:
                nc.scalar.activation(wts[:], wtp[:], Ident)
            else:
                nc.vector.tensor_copy(wts[:], wtp[:])
            return wts

        def stage_exp(scp):
            eT = ep.tile([128, 4, 128], bf16, tag="e", name="eT")
            nc.scalar.activation(eT[:], scp[:].rearrange("p (q n) -> p q n",
                                                         q=4), Exp)
            nc.gpsimd.memset(eT[0:64, :, 64:128], 0.0)
            nc.gpsimd.memset(eT[64:128, :, 0:64], 0.0)
            return eT

        def stage_sm1(eT):
            cst = pp_n.tile([128, 4], f32, tag="nrm", name="cst")
            for q in range(4):
                nc.tensor.matmul(cst[:, q:q + 1], eT[:, q, :], onescol[:],
                                 start=True, stop=True)
            rqt = rp.tile([128, 4], bf16, tag="rqt", name="rqt")
            with nc.allow_low_precision(reason="softmax normalizer in bf16"):
                nc.vector.reciprocal(rqt[:], cst[:])
            return rqt

        def stage_sm2(rqt):
            rqtT = pp_n.tile([1, 512], f32, tag="nrm", name="rqtT")
            for q in range(4):
                nc.tensor.matmul(rqtT[:, q * 128:(q + 1) * 128],
                                 rqt[:, q:q + 1], ident[:],
                                 start=True, stop=True)
            rqs = rp.tile([1, 512], bf16, tag="rqs", name="rqs")
            nc.vector.tensor_copy(rqs[:], rqtT[:])
            return rqs

        def stage_sm3(eT, rqs):
            rbc = pp_n.tile([128, 512], f32, tag="nrm", name="rbc")
            nc.tensor.matmul(rbc[:], onesrow[:], rqs[:],
                             start=True, stop=True)
            pt = ptp.tile([128, 4, 128], bf16, tag="pt", name="pt")
            nc.vector.tensor_tensor(pt[:], eT[:, :, :], rbc[:], Mult)
            return pt

        def stage_Y(st, g, wts_list, pt):
            yp = pp_y.tile([128, 2, 512], f32, tag="yp", name="yp")
            for p in range(4):
                wts = wts_list[p // 2]
                pi = p % 2
                nc.tensor.matmul(blk(yp[:, 0], p), wts[:, pi, 0:128],
                                 pt[:, p, :], start=True, stop=True)
                nc.tensor.matmul(blk(yp[0:64, 1], p), wts[:, pi, 128:192],
                                 pt[:, p, :], start=True, stop=True)
            fr1, fr2 = st["fr"]
            nc.scalar.activation(fr1[:, g], yp[:, 0], Ident)
            nc.vector.tensor_copy(fr2[:, g], yp[0:64, 1])

        # ---- flat cross-band software pipeline, 3 groups deep ----
        # iteration k: sm1(k) | feeder | WC(k+1) h0 | sm2(k) | WC(k+1) h1
        #              | sm3(k) -> pt(k) | exp(k+1) | Y(k-1) | finals(k-1)
        # Chain stages are separated by large independent PE blocks so the
        # strict-FIFO PE never blocks on a fresh cross-engine result.
        NG = n_bands * GPB

        def kb(k):
            return (k // GPB, k % GPB) if 0 <= k < NG else (None, None)

        st_by_band = {0: alloc_band(0)}
        for _ in g_substeps(st_by_band[0]):
            pass
        scp0 = pp_s.tile([128, 512], f32, tag="sp", name="scp")
        wts_cur = [stage_WC_half(st_by_band[0], 0, 0, scp0),
                   stage_WC_half(st_by_band[0], 0, 1, scp0)]
        eT_cur = stage_exp(scp0)
        wts_prev = None
        pt_prev = None
        eT_pend = None  # eT for group k+1 (exp'd at end of iter k)
        feeder = None
        for k in range(NG):
            hk, gk = kb(k)
            h1, g1 = kb(k + 1)
            if gk == 0 and hk + 1 < n_bands:
                st_by_band[hk + 1] = alloc_band(hk + 1)
                feeder = g_substeps(st_by_band[hk + 1])

            rqt = stage_sm1(eT_cur)

            if feeder is not None:
                next(feeder, None)
                if gk == 0 or gk == 2:
                    next(feeder, None)

            if h1 is not None:
                scp_n = pp_s.tile([128, 512], f32, tag="sp", name="scp")
                st1 = st_by_band[h1]
                wts_a = stage_WC_half(st1, g1, 0, scp_n)
            else:
                scp_n = None

            rqs = stage_sm2(rqt)

            if h1 is not None:
                wts_b = stage_WC_half(st1, g1, 1, scp_n)
                wts_next = [wts_a, wts_b]
            else:
                wts_next = None

            pt_cur = stage_sm3(eT_cur, rqs)

            if scp_n is not None:
                eT_pend = stage_exp(scp_n)
            else:
                eT_pend = None

            if k > 0:
                hp, gp_ = kb(k - 1)
                stage_Y(st_by_band[hp], gp_, wts_prev, pt_prev)
                if gp_ == GPB - 1:
                    fb1, fb2 = st_by_band[hp]["fb"]
                    nc.sync.dma_start(out=y[0:128, hp * 8:(hp + 1) * 8, :],
                                      in_=fb1[:])
                    nc.sync.dma_start(out=y[128:192, hp * 8:(hp + 1) * 8, :],
                                      in_=fb2[:])
                    del st_by_band[hp]

            wts_prev, pt_prev = wts_cur, pt_cur
            wts_cur, eT_cur = wts_next, eT_pend

        # epilogue: last group's Y
        hp, gp_ = kb(NG - 1)
        stage_Y(st_by_band[hp], gp_, wts_prev, pt_prev)
        fb1, fb2 = st_by_band[hp]["fb"]
        nc.sync.dma_start(out=y[0:128, hp * 8:(hp + 1) * 8, :], in_=fb1[:])
        nc.sync.dma_start(out=y[128:192, hp * 8:(hp + 1) * 8, :], in_=fb2[:])

    nc.compile()
    return nc


def prep_weights(w_qkv, b_qkv, w_proj, b_proj):
    import ml_dtypes

    bf16 = ml_dtypes.bfloat16
    scale = np.float32(C ** -0.5)
    w_qkv = np.asarray(w_qkv, dtype=np.float32)
    b_qkv = np.asarray(b_qkv, dtype=np.float32)
    w_proj = np.asarray(w_proj, dtype=np.float32)
    b_proj = np.asarray(b_proj, dtype=np.float32)
    wq, wk, wv = w_qkv[0:C], w_qkv[C:2 * C], w_qkv[2 * C:3 * C]
    bq, bk, bv = b_qkv[0:C], b_qkv[C:2 * C], b_qkv[2 * C:3 * C]
    m = scale * (wq.T @ wk)  # S[s,t] = x_s^T M x_t
    alpha = scale * (wk.T @ bq)  # a_t = alpha^T x_t
    gt = np.concatenate([m.T, alpha.reshape(C, 1)], axis=1)  # [192, 193]
    wu = w_proj @ wv
    bpp = b_proj + w_proj @ bv
    wut2 = np.concatenate([wu.T[128:192], bpp.reshape(1, C)], axis=0)
    return {
        "eye": np.eye(128, dtype=np.float32).astype(bf16),
        "wgk1": np.ascontiguousarray(gt[0:128]).astype(bf16),
        "wgk2": np.ascontiguousarray(gt[128:192]).astype(bf16),
        "wut1": np.ascontiguousarray(wu.T[0:128]).astype(bf16),
        "wut2": np.ascontiguousarray(wut2).astype(bf16),
    }


_PROGRAM_CACHE = {}


def get_program(n_bands, width=256):
    key = (n_bands, width)
    if key not in _PROGRAM_CACHE:
        _PROGRAM_CACHE[key] = build_program(n_bands, width)
    return _PROGRAM_CACHE[key]


def make_in_maps(x, w_qkv, b_qkv, w_proj, b_proj):
    import ml_dtypes

    x = np.asarray(x, dtype=np.float32)
    B, c, H, W = x.shape
    nb, nw = H // WS, W // WS
    # window-major token order: [C, band, window, i, j]
    xwm = x.reshape(B, c, nb, WS, nw, WS).transpose(0, 1, 2, 4, 3, 5)
    xwm = np.ascontiguousarray(xwm.reshape(B, c, nb, nw * S)).astype(
        ml_dtypes.bfloat16)
    wts = prep_weights(w_qkv, b_qkv, w_proj, b_proj)
    return [{"x": xwm[b], **wts} for b in range(B)]


def assemble_output(results):
    out = np.stack([results[b]["y"] for b in range(len(results))], axis=0)
    return out.astype(np.float32)


def kernel(x, w_qkv, b_qkv, w_proj, b_proj):
    from concourse.bass_utils import run_bass_kernel_spmd

    x = np.asarray(x, dtype=np.float32)
    B, c, H, W = x.shape
    assert c == C
    nc = get_program(H // WS, W)
    in_maps = make_in_maps(x, w_qkv, b_qkv, w_proj, b_proj)
    res = run_bass_kernel_spmd(nc, in_maps, core_ids=list(range(B)))
    return assemble_output(res.results)
